# revision 2
# baseline (speedup 1.0000x reference)
"""AtomwiseReadout segment-reduce kernel for 8 TRN2 NeuronCores.

reference computation:
    atomwise = f @ w_e + z_bias[z]            # [N, 1]
    e_total  = segment_sum(atomwise, seg)     # [B, 1], 20 atoms per molecule

Strategy (pure data-parallel, no collectives):
  - atoms are sharded contiguously at molecule boundaries across 8 cores;
    shards overlap slightly so every core processes a uniform 98 "supers"
    of 2560 atoms (= 128 molecules); duplicated molecules are dropped on
    the host at gather time.
  - per super (2560 atoms), on-device:
      f tile        = one f32->bf16 cast-DMA (SWDGE)
      S[mol, feat]  = A_t.T @ f_tiles   (5 accumulating bf16 matmuls, A_t
                                         are constant 0/1 segment matrices)
      z_rep         = z broadcast across one-hot width (Scalar engine)
      oh[atom, v]   = (z_rep == iota)   (one packed-bf16 DVE is_equal, 2x)
      hist[mol, v]  = A_t.T @ oh_tiles  (5 accumulating bf16 matmuls)
      e[mol]        = segmented_reduce([hist * z_bias | S * w_e])
                      (two DVE mults into one scratch + one DVE reduce)
"""

import numpy as np
import ml_dtypes

import concourse.bass as bass
import concourse.bacc as bacc
import concourse.mybir as mybir
import concourse.tile as tile
from concourse.bass_utils import run_bass_kernel_spmd


def _ensure_ntff_hook():
    """Restore the NTFF profile hook if the image's antenv lacks axon_hooks.

    trn_boot.boot() registers this hook at interpreter start, but degrades
    silently when ``antenv.axon_hooks`` is missing — and bass_utils then
    crashes on the import when trace=True. Recreate the module with the
    same hook boot() would have installed. No-op when the real module
    exists.
    """
    try:
        import antenv.axon_hooks  # noqa: F401

        return
    except ImportError:
        pass
    try:
        import sys
        import types

        from trn_agent_boot.trn_boot import _ntff_profile_via_ctypes

        hook = _ntff_profile_via_ctypes("/opt/axon/libaxon_pjrt.so")
        mod = types.ModuleType("antenv.axon_hooks")
        mod.get_axon_ntff_profile_hook = lambda: hook
        mod.set_axon_ntff_profile_hook = lambda h: None
        sys.modules["antenv.axon_hooks"] = mod
    except Exception:
        pass


_ensure_ntff_hook()
# problem constants (hardcoded per spec)
N_ATOMS = 2_000_000
N_MOL = 100_000
APM = 20          # atoms per molecule
D = 128           # feature dim
V = 86            # z vocabulary (0..85)
N_CORES = 8

# tiling
P = 128                    # partitions / atoms per subtile
SUBT_PER_SUP = 20          # subtiles per super  (= lcm(128,20)*4 / 128)
SUP_ATOMS = P * SUBT_PER_SUP      # 2560 atoms per super
SUP_MOLS = SUP_ATOMS // APM       # 128 molecules per super
N_SUP = 98                 # supers per core
SHARD_ATOMS = N_SUP * SUP_ATOMS   # 250880
SHARD_MOLS = SHARD_ATOMS // APM   # 12544
MOLS_PER_CORE = N_MOL // N_CORES  # 12500 molecules each core is responsible for

F32 = mybir.dt.float32
F32R = mybir.dt.float32r
BF16 = mybir.dt.bfloat16

# float32r gives full-rate f32 matmuls on the PE at N>=256.  Set False to
# fall back to plain (4x slower) fp32 matmuls.
USE_F32R = True
TRACE = False  # test harness can flip this to get a profile


def _seg_matrices():
    """A[p, t*32+m] = 1 iff atom (t*128+p) of a 640-atom block belongs to
    molecule m of that block (m in 0..31)."""
    A = np.zeros((P, 5, 32), np.float32)
    for t in range(5):
        for p in range(P):
            A[p, t, (t * P + p) // APM] = 1.0
    return np.ascontiguousarray(A.reshape(P, 5 * 32))


def build(nc, n_sup=N_SUP):
    shard_atoms = n_sup * SUP_ATOMS
    n_sub = shard_atoms // P

    f = nc.dram_tensor("f", [shard_atoms, D], F32, kind="ExternalInput")
    zc = nc.dram_tensor("z_cols", [P, n_sub], BF16, kind="ExternalInput")
    a16 = nc.dram_tensor("a_bf16", [P, 160], BF16, kind="ExternalInput")
    iota = nc.dram_tensor("iota", [P, 2 * SUBT_PER_SUP * V], BF16, kind="ExternalInput")
    w = nc.dram_tensor("w_rep", [P, 4 * D], F32, kind="ExternalInput")
    zb = nc.dram_tensor("zb_rep", [P, 4 * V], F32, kind="ExternalInput")
    # 3 supers share one PSUM tile (out base partition must be 0/32/64)
    ncyc = (n_sup + 2) // 3
    out = nc.dram_tensor("out", [96, 4 * ncyc], F32, kind="ExternalOutput")

    # atom row = n*2560 + s*128 + p  ->  [n, p, s, d]
    fv = f.ap().rearrange("(n s p) d -> n p s d", s=SUBT_PER_SUP, p=P)

    with tile.TileContext(nc) as tc:
        with (
            tc.tile_pool(name="const", bufs=1) as cpool,
            tc.tile_pool(name="fpool", bufs=10) as fpool,
            tc.tile_pool(name="work", bufs=4) as pool,
            tc.tile_pool(name="psum_s", bufs=4, space="PSUM") as ppool_s,
            tc.tile_pool(name="psum_h", bufs=4, space="PSUM") as ppool_h,
        ):
            zc_sb = cpool.tile([P, n_sub], BF16)
            nc.sync.dma_start(out=zc_sb[:], in_=zc.ap())
            a16_sb = cpool.tile([P, 160], BF16)
            nc.sync.dma_start(out=a16_sb[:], in_=a16.ap())
            iota_sb = cpool.tile([P, 2 * SUBT_PER_SUP * V], BF16)
            nc.sync.dma_start(out=iota_sb[:], in_=iota.ap())
            w_sb = cpool.tile([P, 4 * D], F32)
            nc.sync.dma_start(out=w_sb[:], in_=w.ap())
            zb_sb = cpool.tile([P, 4 * V], F32)
            nc.sync.dma_start(out=zb_sb[:], in_=zb.ap())

            # DVE-local copies of every tile a DVE instruction reads, so
            # those instructions never need DMA-semaphore waits (the
            # core_v3 TT/TTR encodings have very few sync-wait slots).
            zc2 = cpool.tile([P, n_sub], BF16)
            nc.vector.tensor_copy(out=zc2[:], in_=zc_sb[:])
            iota2 = cpool.tile([P, 2 * SUBT_PER_SUP * V], BF16)
            nc.vector.tensor_copy(out=iota2[:], in_=iota_sb[:])
            w2 = cpool.tile([P, 4 * D], F32)
            nc.vector.tensor_copy(out=w2[:], in_=w_sb[:])
            zb2 = cpool.tile([P, 4 * V], F32)
            nc.vector.tensor_copy(out=zb2[:], in_=zb_sb[:])
            w4 = w2[:]
            zb4 = zb2[:]

            for n in range(n_sup):
                # one cast-DMA per super: f32 in HBM -> bf16 in SBUF
                f_sb = fpool.tile([P, SUP_ATOMS], BF16, tag="f")
                nc.gpsimd.dma_start(out=f_sb[:], in_=fv[n])
                fsub = f_sb[:].rearrange("p (s d) -> p s d", d=D)
                # one-hot of z for this super, laid out (t, g, v);
                # z_cols is host-permuted so column n*20+j holds subtile
                # s = t+5g with j = t*4+g
                oh = pool.tile([P, SUBT_PER_SUP * V], BF16, tag="oh")
                z_ap = (
                    zc2[:, n * SUBT_PER_SUP : (n + 1) * SUBT_PER_SUP]
                    .unsqueeze(2)
                    .to_broadcast([P, SUBT_PER_SUP, V])
                )
                # Scalar engine replicates z across the one-hot width so the
                # DVE compare below is fully packed bf16 (2x perf mode)
                z_rep = pool.tile([P, SUBT_PER_SUP * V], BF16, tag="zr")
                nc.scalar.copy(out=z_rep[:], in_=z_ap)
                nc.vector.tensor_tensor(
                    out=oh[:],
                    in0=iota2[:, : SUBT_PER_SUP * V],
                    in1=z_rep[:],
                    op=mybir.AluOpType.is_equal,
                )

                # supers land in 32-partition bands of a shared PSUM tile
                # (matmul tile_position comes from the out AP base partition)
                if n % 3 == 0:
                    s_ps = ppool_s.tile([96, 512], F32, tag="S")
                    # padded to a full PSUM bank so every buf stays aligned
                    h_ps_full = ppool_h.tile([96, 512], F32, tag="H")
                    h_ps = h_ps_full[:, : 4 * V]
                band = slice((n % 3) * 32, (n % 3) * 32 + 32)
                for t in range(5):
                    nc.tensor.matmul(
                        out=s_ps[band, :],
                        lhsT=a16_sb[:, t * 32 : (t + 1) * 32],
                        rhs=fsub[:, t::5, :],
                        start=(t == 0),
                        stop=(t == 4),
                    )
                    nc.tensor.matmul(
                        out=h_ps[band, :],
                        lhsT=a16_sb[:, t * 32 : (t + 1) * 32],
                        rhs=oh[:, t * 4 * V : (t + 1) * 4 * V],
                        start=(t == 0),
                        stop=(t == 4),
                    )

                if n % 3 == 2 or n == n_sup - 1:
                    nb = (n % 3) + 1
                    pr = slice(0, nb * 32)
                    scr = pool.tile([96, 4 * (V + D)], F32, tag="scr")
                    sv = scr[:].rearrange("p (g c) -> p g c", g=4)
                    nc.vector.tensor_tensor(
                        out=sv[pr, :, :V],
                        in0=h_ps[pr, :],
                        in1=zb4[pr, :],
                        op=mybir.AluOpType.mult,
                    )
                    nc.vector.tensor_tensor(
                        out=sv[pr, :, V:],
                        in0=s_ps[pr, :],
                        in1=w4[pr, :],
                        op=mybir.AluOpType.mult,
                    )
                    eo = pool.tile([96, 4], F32, tag="eo")
                    nc.vector.tensor_reduce(
                        out=eo[pr, :],
                        in_=sv[pr],
                        axis=mybir.AxisListType.X,
                        op=mybir.AluOpType.add,
                    )
                    c = n // 3
                    nc.sync.dma_start(
                        out=out.ap()[: nb * 32, c * 4 : (c + 1) * 4],
                        in_=eo[pr, :],
                    )
    nc.compile()
    return nc


_ZPERM = np.array([t * 4 + g for g in range(4) for t in range(5)]).argsort()
# _ZPERM[j] = subtile s = t+5g for j = t*4+g


def _prep_core_inputs(f, z, w_e, z_bias, start, shard_atoms=SHARD_ATOMS):
    """Per-core input map. f/z are the full arrays; start = first atom row."""
    n_sup = shard_atoms // SUP_ATOMS
    zs = np.asarray(z[start : start + shard_atoms]).astype(np.float32)
    zs = zs.reshape(n_sup, SUBT_PER_SUP, P)[:, _ZPERM, :]
    z_cols = np.ascontiguousarray(
        zs.reshape(n_sup * SUBT_PER_SUP, P).T.astype(ml_dtypes.bfloat16)
    )
    return {
        "f": f[start : start + shard_atoms],
        "z_cols": z_cols,
        "a_bf16": _A_BF16,
        "iota": _IOTA,
        "w_rep": np.ascontiguousarray(
            np.broadcast_to(
                np.tile(np.asarray(w_e, np.float32).reshape(D), 4), (P, 4 * D)
            )
        ),
        "zb_rep": np.ascontiguousarray(
            np.broadcast_to(
                np.tile(np.asarray(z_bias, np.float32).reshape(V), 4), (P, 4 * V)
            )
        ),
    }


_A_F32 = _seg_matrices()
_A_BF16 = _A_F32.astype(ml_dtypes.bfloat16)
_IOTA = np.ascontiguousarray(
    np.broadcast_to(
        np.tile(np.arange(V, dtype=np.float32), 2 * SUBT_PER_SUP),
        (P, 2 * SUBT_PER_SUP * V),
    )
).astype(ml_dtypes.bfloat16)

_NC_CACHE = {}
_LAST_RESULTS = None  # BassKernelResults of the most recent run (for profiling)


def kernel(z, f, num_atoms, w_e, z_bias):
    global _LAST_RESULTS
    z = np.asarray(z)
    f = np.ascontiguousarray(np.asarray(f, dtype=np.float32))
    w_e = np.asarray(w_e, dtype=np.float32)
    z_bias = np.asarray(z_bias, dtype=np.float32)
    assert f.shape == (N_ATOMS, D)

    key = ("full", N_SUP, USE_F32R)
    if key not in _NC_CACHE:
        _NC_CACHE[key] = build(bacc.Bacc(), N_SUP)
    nc = _NC_CACHE[key]

    # core i handles molecules [i*12500, (i+1)*12500); its shard starts at
    # atom i*250000 except the last core, whose shard is right-aligned so
    # no padding is ever needed.
    starts = [i * MOLS_PER_CORE * APM for i in range(N_CORES - 1)]
    starts.append(N_ATOMS - SHARD_ATOMS)
    in_maps = [_prep_core_inputs(f, z, w_e, z_bias, s) for s in starts]

    res = run_bass_kernel_spmd(nc, in_maps, core_ids=list(range(N_CORES)), trace=TRACE)
    _LAST_RESULTS = res

    ncyc = (N_SUP + 2) // 3
    out = np.empty((N_MOL, 1), np.float32)
    for i in range(N_CORES):
        # device layout: out[b*32+m, c*4+g] = e of mol (c*3+b)*128 + g*32 + m
        arr = np.asarray(res.results[i]["out"])  # [96, 4*ncyc]
        e = (
            arr.reshape(3, 32, ncyc, 4)
            .transpose(2, 0, 3, 1)
            .reshape(ncyc * 3 * SUP_MOLS)[:SHARD_MOLS]
        )
        first_mol = starts[i] // APM
        lo = i * MOLS_PER_CORE
        out[lo : lo + MOLS_PER_CORE, 0] = e[lo - first_mol : lo - first_mol + MOLS_PER_CORE]
    return out

